# revision 1
# baseline (speedup 1.0000x reference)
"""Trainium2 Bass kernel for nn_BoundLoss (pull/push embedding loss, segment_reduce).

Strategy: pure data parallel, 1 image per NeuronCore (B=8, 8 cores).
All reductions on device. Output per core: (loss_pull, loss_push) scalars.

Key ideas:
  - Segment sums (by gt_kernels / gt_texts, M=16 ids) via block-diagonal
    one-hot matmuls on the tensor engine: J pixel-column groups share one
    stationary-weight load; off-diagonal products land in PSUM cells we
    never read.
  - The per-pixel gather of centroid stats G[tt[n]] is folded into a single
    stationary-weight matmul computing, for every pixel and every id m,
    z_m = s2 - 2*dot(sim, G[m]) + g2[m]  (a "D-hat" tensor), using a
    block-diagonal G-matrix with 8 pixel sub-row slots; per-pixel selection
    of the right m is 16 mask-mult-accumulate passes on the vector engine.
  - l = log1p(relu(sqrt(z)-0.5)^2) chain on the scalar engine.
"""

import os
import numpy as np
from contextlib import ExitStack

EPS = 1e-12

FULL_CFG = dict(H=640, W=640)

_CACHE = {}


def _cfg(H, W):
    P = 128
    N = H * W
    F = N // P
    assert F * P == N
    if F % 400 == 0 and F >= 1600:
        FC = 400
    else:
        FC = F // 4 if F % 4 == 0 and F // 4 <= 512 else F
        if FC > 512:
            raise ValueError("bad FC")
    NQ = max(1, F // (2 * FC))
    FQ = F // NQ
    assert FQ % FC == 0 and F % FQ == 0
    OHC = min(F, 400)
    assert F % OHC == 0
    # PE group sizes (pixel columns per stationary-weight load)
    JK = 25 if OHC % 25 == 0 else 8   # kt family: 5 ch -> 125 weight cols
    JT = 32 if FQ % 32 == 0 else 8    # tt family: 2 ch -> 64 weight cols
    assert OHC % JK == 0 and FQ % JT == 0
    return dict(H=H, W=W, P=P, N=N, F=F, FC=FC, FQ=FQ, NQ=NQ, OHC=OHC,
                JK=JK, JT=JT, M=16)


def build(cfg, for_sim=False):
    import concourse.bass as bass
    import concourse.bacc as bacc
    import concourse.tile as tile
    from concourse import mybir

    dt = mybir.dt
    Alu = mybir.AluOpType
    Act = mybir.ActivationFunctionType
    AX = mybir.AxisListType

    P, F, M = cfg["P"], cfg["F"], cfg["M"]
    FC, FQ, NQ, OHC = cfg["FC"], cfg["FQ"], cfg["NQ"], cfg["OHC"]
    JK, JT = cfg["JK"], cfg["JT"]

    nc = bacc.Bacc("TRN2", target_bir_lowering=False, debug=for_sim)

    sim_d = nc.dram_tensor("sim", [4, P, F], dt.float32, kind="ExternalInput")
    kt_d = nc.dram_tensor("kt", [P, F], dt.int32, kind="ExternalInput")
    tt_d = nc.dram_tensor("tt", [P, F], dt.int32, kind="ExternalInput")
    ident_d = nc.dram_tensor("ident16", [16, 16], dt.float32, kind="ExternalInput")
    iu_d = nc.dram_tensor("iu16", [16, 16], dt.float32, kind="ExternalInput")
    mge1_d = nc.dram_tensor("mge1", [16, 1], dt.float32, kind="ExternalInput")
    e6_d = nc.dram_tensor("e6", [6, 48], dt.bfloat16, kind="ExternalInput")
    dmask_d = nc.dram_tensor("dmask48", [48, 128], dt.bfloat16,
                             kind="ExternalInput")
    dselk_d = nc.dram_tensor("dselk", [JK * 5, 16 * JK], dt.bfloat16,
                             kind="ExternalInput")
    c5_d = nc.dram_tensor("c5", [JK * 5, 5], dt.float32, kind="ExternalInput")
    dselt_d = nc.dram_tensor("dselt", [JT * 2, 16 * JT], dt.bfloat16,
                             kind="ExternalInput")
    c2_d = nc.dram_tensor("c2", [JT * 2, 2], dt.float32, kind="ExternalInput")
    out_d = nc.dram_tensor("out", [1, 2], dt.float32, kind="ExternalOutput")

    with ExitStack() as ctx:
        tc = ctx.enter_context(tile.TileContext(nc, trace_sim=for_sim))

        big = ctx.enter_context(tc.tile_pool(name="big", bufs=1))
        t16 = ctx.enter_context(tc.tile_pool(name="t16", bufs=1))
        pst = ctx.enter_context(tc.tile_pool(name="pst", bufs=2, space="PSUM"))

        _tiny_n = [0]

        def tiny_ps(shape):
            _tiny_n[0] += 1
            return pst.tile(shape, dt.float32, tag="tiny",
                            name=f"tinyps{_tiny_n[0]}")

        # ---- constants ----
        ident16 = big.tile([16, 16], dt.float32)
        nc.sync.dma_start(out=ident16[:], in_=ident_d.ap())
        iu16 = big.tile([16, 16], dt.float32)
        nc.sync.dma_start(out=iu16[:], in_=iu_d.ap())
        mge1 = t16.tile([16, 1], dt.float32)
        nc.sync.dma_start(out=mge1[:], in_=mge1_d.ap())
        e6 = big.tile([6, 48], dt.bfloat16)
        nc.sync.dma_start(out=e6[:], in_=e6_d.ap())
        dmask48 = big.tile([48, 128], dt.bfloat16)
        nc.sync.dma_start(out=dmask48[:], in_=dmask_d.ap())
        dselk = big.tile([JK * 5, 16 * JK], dt.bfloat16)
        nc.sync.dma_start(out=dselk[:], in_=dselk_d.ap())
        c5 = big.tile([JK * 5, 5], dt.float32)
        nc.sync.dma_start(out=c5[:], in_=c5_d.ap())
        dselt = big.tile([JT * 2, 16 * JT], dt.bfloat16)
        nc.sync.dma_start(out=dselt[:], in_=dselt_d.ap())
        c2 = big.tile([JT * 2, 2], dt.float32)
        nc.sync.dma_start(out=c2[:], in_=c2_d.ap())
        ones16 = t16.tile([16, 1], dt.float32)
        nc.vector.memset(ones16[:], 1.0)
        b_eps = t16.tile([128, 1], dt.float32)
        nc.vector.memset(b_eps[:], EPS)
        b_mhalf = t16.tile([128, 1], dt.float32)
        nc.vector.memset(b_mhalf[:], -0.5)
        b_three = t16.tile([128, 1], dt.float32)
        nc.vector.memset(b_three[:], 3.0)

        # ---- persistent planes ----
        vplI = big.tile([P, F, 5], dt.bfloat16)   # (sim0..3, ones) interleaved
        simbf4 = big.tile([P, 4, F], dt.bfloat16)  # plane-major (for remaps)
        vpl2I = big.tile([P, F, 2], dt.bfloat16)  # (l, ones) interleaved
        s2bf = big.tile([P, F], dt.bfloat16)
        ttbf = big.tile([P, F], dt.bfloat16)
        gblock = big.tile([48, 128], dt.bfloat16)
        ktp_ctx = ExitStack()
        ktp = ktp_ctx.enter_context(tc.tile_pool(name="ktp", bufs=1))
        ktbf = ktp.tile([P, F], dt.bfloat16)

        nc.gpsimd.memset(vplI[:, :, 4], 1.0)
        nc.gpsimd.memset(vpl2I[:, :, 1], 1.0)

        # ---- load + convert inputs; s2 ----
        with tc.tile_pool(name="ld", bufs=2) as ld:
            kti = ld.tile([P, F], dt.int32, tag="idx")
            nc.sync.dma_start(out=kti[:], in_=kt_d.ap())
            nc.vector.tensor_copy(out=ktbf[:], in_=kti[:])
            tti = ld.tile([P, F], dt.int32, tag="idx")
            nc.sync.dma_start(out=tti[:], in_=tt_d.ap())
            nc.vector.tensor_copy(out=ttbf[:], in_=tti[:])
            for c in range(4):
                sf = ld.tile([P, F], dt.float32, tag="simf")
                nc.sync.dma_start(out=sf[:], in_=sim_d.ap()[c])
                nc.vector.tensor_copy(out=simbf4[:, c, :], in_=sf[:])
                eng = nc.gpsimd if c % 2 == 0 else nc.vector
                eng.tensor_copy(out=vplI[:, :, c], in_=sf[:])
                if c == 0:
                    nc.scalar.activation(out=s2bf[:], in_=sf[:], func=Act.Square)
                else:
                    sq = ld.tile([P, F], dt.bfloat16, tag="sq")
                    nc.scalar.activation(out=sq[:], in_=sf[:], func=Act.Square)
                    nc.vector.tensor_add(s2bf[:], s2bf[:], sq[:])

        # ---- phase 1: kernel-id segment sums ----
        # stationary = vplI group [128, JK*5] (contiguous); moving = onehot
        # [128, (m, J')]; psum [(J,c), (m,J')]; diagonal J==J' is wanted.
        NGK = F // JK
        skC_ps = tiny_ps([5, 16 * JK])
        with tc.tile_pool(name="ps1", bufs=1, space="PSUM") as psum1:
            p1 = psum1.tile([JK * 5, 16 * JK], dt.float32)
            with tc.tile_pool(name="ohk", bufs=2) as ohkp:
                for blk in range(F // OHC):
                    ohk = ohkp.tile([P, M, OHC], dt.bfloat16, tag="ohk")
                    for m in range(M):
                        nc.vector.tensor_scalar(
                            out=ohk[:, m, :],
                            in0=ktbf[:, blk * OHC:(blk + 1) * OHC],
                            scalar1=float(m), scalar2=None, op0=Alu.is_equal)
                    for t in range(OHC // JK):
                        g = blk * OHC + t * JK
                        lhsT = vplI[:, g:g + JK, :]
                        rhs = ohk[:, :, t * JK:(t + 1) * JK]
                        nc.tensor.matmul(p1[:], lhsT, rhs,
                                         start=(g == 0), stop=(g == F - JK))
            # diagonal extraction: mask off-diag, sum rows per channel via
            # matmul, then strided-reduce over J'.
            p1m = big.tile([JK * 5, 16 * JK], dt.float32)
            nc.vector.tensor_mul(p1m[:], p1[:], dselk[:])
            nc.tensor.matmul(skC_ps[:], c5[:], p1m[:], start=True, stop=True)
        skC = big.tile([5, 16], dt.float32)
        nc.vector.tensor_reduce(
            out=skC[:],
            in_=skC_ps.rearrange("p (m j) -> p m j", j=JK),
            axis=AX.X, op=Alu.add)
        skT_ps = tiny_ps([16, 5])
        nc.tensor.transpose(skT_ps[:], skC[:], ident16[0:5, 0:5])
        sk = big.tile([16, 5], dt.float32)
        nc.vector.tensor_copy(out=sk[:], in_=skT_ps[:])
        ktp_ctx.close()

        # ---- stats: G, g2, valid, Ghat-block ----
        cntk_c = t16.tile([16, 1], dt.float32)
        nc.vector.tensor_scalar(out=cntk_c[:], in0=sk[:, 4:5], scalar1=1.0,
                                scalar2=None, op0=Alu.max)
        rck = t16.tile([16, 1], dt.float32)
        nc.vector.reciprocal(rck[:], cntk_c[:])
        G = t16.tile([16, 4], dt.float32)
        nc.vector.tensor_scalar(out=G[:], in0=sk[:, 0:4], scalar1=rck[:, 0:1],
                                scalar2=None, op0=Alu.mult)
        gsq = t16.tile([16, 4], dt.float32)
        nc.scalar.activation(out=gsq[:], in_=G[:], func=Act.Square)
        g2 = t16.tile([16, 1], dt.float32)
        nc.vector.tensor_reduce(out=g2[:], in_=gsq[:], axis=AX.X, op=Alu.add)

        gext = t16.tile([16, 6], dt.float32)
        nc.vector.tensor_scalar(out=gext[:, 0:4], in0=G[:], scalar1=-2.0,
                                scalar2=None, op0=Alu.mult)
        nc.vector.tensor_copy(out=gext[:, 4:5], in_=g2[:])
        nc.vector.memset(gext[:, 5:6], 1.0)

        gsT = tiny_ps([6, 16])
        nc.tensor.transpose(gsT[:], gext[:], ident16[:])
        gsb = big.tile([6, 16], dt.bfloat16)
        nc.vector.tensor_copy(out=gsb[:], in_=gsT[:])
        # gblock[ch*8+i, m*8+i'] = delta(i,i') * Ghat[ch, m]:
        # broadcast Ghat across 8 col-slots (DVE), expand partition groups
        # via E6 matmul, then mask the slot diagonal.
        gbig_row = big.tile([6, 128], dt.bfloat16)
        ga = gsb[:]
        gbc_ap = bass.AP(tensor=ga.tensor, offset=ga.offset,
                         ap=[list(ga.ap[0]), list(ga.ap[1]), [0, 8]])
        nc.vector.tensor_copy(out=gbig_row[:], in_=gbc_ap)
        gbig_ps = tiny_ps([48, 128])
        nc.tensor.matmul(gbig_ps[:], e6[:], gbig_row[:], start=True, stop=True)
        nc.vector.tensor_mul(gblock[:], gbig_ps[:], dmask48[:])

        vg = t16.tile([16, 1], dt.float32)
        nc.vector.tensor_scalar(out=vg[:], in0=sk[:, 4:5], scalar1=0.0,
                                scalar2=None, op0=Alu.is_gt)
        valid = t16.tile([16, 1], dt.float32)
        nc.vector.tensor_mul(valid[:], vg[:], mge1[:])

        # ---- phase 2: per-quarter D-hat, resolve, l, PE-tt ----
        NGT = FQ // JT
        psum3 = ctx.enter_context(tc.tile_pool(name="ps3", bufs=1, space="PSUM"))
        p3 = psum3.tile([JT * 2, 16 * JT], dt.float32)
        onesq = big.tile([P, FQ], dt.bfloat16)
        nc.gpsimd.memset(onesq[:], 1.0)
        zfull = big.tile([P, F], dt.bfloat16)
        # ---- loop 1: D-hat + resolve -> zfull ----
        with tc.tile_pool(name="q_sb", bufs=1) as qp, \
             tc.tile_pool(name="q_sb2", bufs=2) as qp2, \
             tc.tile_pool(name="q_ps", bufs=3, space="PSUM") as qps, \
             tc.tile_pool(name="q_z", bufs=2) as zp:
            for q in range(NQ):
                qlo = q * FQ
                dstage = qp.tile([P, 16, FQ], dt.bfloat16, tag="dstage")
                for sh in range(FQ // FC):
                    slo = qlo + sh * FC
                    # rhs layout [48 = ch*8+i, 16u, FC]
                    rhs_h = qp2.tile([48, 16, FC], dt.bfloat16, tag="rhs")
                    for ch in range(6):
                        src = (simbf4[:, ch, slo:slo + FC] if ch < 4 else
                               (onesq[:, 0:FC] if ch == 4 else
                                s2bf[:, slo:slo + FC]))
                        nc.sync.dma_start(
                            out=rhs_h[ch * 8:(ch + 1) * 8, :, :], in_=src)
                    for u in range(16):
                        pd = qps.tile([128, FC], dt.float32, tag="pd")
                        nc.tensor.matmul(pd[:], gblock[:], rhs_h[:, u, :],
                                         start=True, stop=True)
                        nc.scalar.activation(
                            out=dstage[:, u, sh * FC:(sh + 1) * FC],
                            in_=pd[:], func=Act.Copy)
                dq = qp.tile([P, M, FQ], dt.bfloat16, tag="dq")
                for m in range(M):
                    nc.sync.dma_start(out=dq[:, m, :],
                                      in_=dstage[m * 8:(m + 1) * 8, :, :])

                # z = sum_m (tt==m) * dq_m  (disjoint masks, fused is_equal)
                zwin = zfull[:, qlo:qlo + FQ]
                twin = ttbf[:, qlo:qlo + FQ]
                nc.vector.scalar_tensor_tensor(
                    out=zwin, in0=twin, scalar=0.0, in1=dq[:, 0, :],
                    op0=Alu.is_equal, op1=Alu.mult)
                for m in range(1, M):
                    pm_t = zp.tile([P, FQ], dt.bfloat16, tag="pm")
                    nc.vector.scalar_tensor_tensor(
                        out=pm_t[:], in0=twin, scalar=float(m),
                        in1=dq[:, m, :], op0=Alu.is_equal, op1=Alu.mult)
                    nc.vector.tensor_add(zwin, zwin, pm_t[:])

        # ---- loop 2: masks, l-chain, PE-tt ----
        with tc.tile_pool(name="l2_sb", bufs=2) as l2p, \
             tc.tile_pool(name="l2_z", bufs=2) as zp2:
            for q in range(NQ):
                qlo = q * FQ
                ohtt = l2p.tile([P, M, FQ], dt.bfloat16, tag="ohtt")
                for m in range(M):
                    nc.vector.tensor_scalar(
                        out=ohtt[:, m, :], in0=ttbf[:, qlo:qlo + FQ],
                        scalar1=float(m), scalar2=None, op0=Alu.is_equal)

                t0 = zp2.tile([P, FQ], dt.bfloat16, tag="lt0")
                nc.scalar.activation(out=t0[:], in_=zfull[:, qlo:qlo + FQ],
                                     func=Act.Relu)
                t1 = zp2.tile([P, FQ], dt.bfloat16, tag="lt1")
                nc.scalar.activation(out=t1[:], in_=t0[:], func=Act.Sqrt,
                                     bias=b_eps[:, 0:1])
                t2 = zp2.tile([P, FQ], dt.bfloat16, tag="lt0")
                nc.scalar.activation(out=t2[:], in_=t1[:], func=Act.Relu,
                                     bias=b_mhalf[:, 0:1])
                t3 = zp2.tile([P, FQ], dt.bfloat16, tag="lt1")
                nc.scalar.activation(out=t3[:], in_=t2[:], func=Act.Square)
                nc.scalar.activation(out=vpl2I[:, qlo:qlo + FQ, 0], in_=t3[:],
                                     func=Act.Ln, bias=1.0)

                for t in range(NGT):
                    g = qlo + t * JT
                    lhsT = vpl2I[:, g:g + JT, :]
                    rhs = ohtt[:, :, t * JT:(t + 1) * JT]
                    nc.tensor.matmul(p3[:], lhsT, rhs,
                                     start=(g == 0), stop=(g == F - JT))

        # tt-family diagonal extraction (same scheme as phase 1)
        p3m = big.tile([JT * 2, 16 * JT], dt.float32)
        nc.vector.tensor_mul(p3m[:], p3[:], dselt[:])
        stC_ps = tiny_ps([2, 16 * JT])
        nc.tensor.matmul(stC_ps[:], c2[:], p3m[:], start=True, stop=True)
        stC = big.tile([2, 16], dt.float32)
        nc.vector.tensor_reduce(
            out=stC[:],
            in_=stC_ps.rearrange("p (m j) -> p m j", j=JT),
            axis=AX.X, op=Alu.add)
        stT_ps = tiny_ps([16, 2])
        nc.tensor.transpose(stT_ps[:], stC[:], ident16[0:2, 0:2])
        st = big.tile([16, 2], dt.float32)
        nc.vector.tensor_copy(out=st[:], in_=stT_ps[:])

        # ---- pull loss ----
        cntt_c = t16.tile([16, 1], dt.float32)
        nc.vector.tensor_scalar(out=cntt_c[:], in0=st[:, 1:2], scalar1=1.0,
                                scalar2=None, op0=Alu.max)
        rct = t16.tile([16, 1], dt.float32)
        nc.vector.reciprocal(rct[:], cntt_c[:])
        pim = t16.tile([16, 1], dt.float32)
        nc.vector.tensor_mul(pim[:], st[:, 0:1], rct[:])
        nc.vector.tensor_mul(pim[:], pim[:], valid[:])

        num_ps = tiny_ps([1, 1])
        nc.tensor.matmul(num_ps[:], pim[:], ones16[:], start=True, stop=True)
        nv_ps = tiny_ps([1, 1])
        nc.tensor.matmul(nv_ps[:], valid[:], ones16[:], start=True, stop=True)
        num_s = t16.tile([1, 1], dt.float32)
        nc.vector.tensor_copy(out=num_s[:], in_=num_ps[:])
        nv_s = t16.tile([1, 1], dt.float32)
        nc.vector.tensor_copy(out=nv_s[:], in_=nv_ps[:])

        nv_c = t16.tile([1, 1], dt.float32)
        nc.vector.tensor_scalar(out=nv_c[:], in0=nv_s[:], scalar1=1.0,
                                scalar2=None, op0=Alu.max)
        rnv = t16.tile([1, 1], dt.float32)
        nc.vector.reciprocal(rnv[:], nv_c[:])
        lpull = t16.tile([1, 1], dt.float32)
        nc.vector.tensor_mul(lpull[:], num_s[:], rnv[:])

        # ---- push loss ----
        ones1x16 = big.tile([1, 16], dt.float32)
        nc.vector.memset(ones1x16[:], 1.0)
        gT_ps = tiny_ps([4, 16])
        nc.tensor.transpose(gT_ps[:], G[:], ident16[:])
        gt_sb = big.tile([4, 16], dt.float32)
        nc.vector.tensor_copy(out=gt_sb[:], in_=gT_ps[:])
        g2r_ps = tiny_ps([1, 16])
        nc.tensor.transpose(g2r_ps[:], g2[:], ident16[:])
        g2row = big.tile([1, 16], dt.float32)
        nc.vector.tensor_copy(out=g2row[:], in_=g2r_ps[:])
        mgt2 = big.tile([4, 16], dt.float32)
        nc.vector.tensor_scalar(out=mgt2[:], in0=gt_sb[:], scalar1=-2.0,
                                scalar2=None, op0=Alu.mult)
        dk2_ps = tiny_ps([16, 16])
        nc.tensor.matmul(dk2_ps[:], mgt2[:], gt_sb[:], start=True, stop=False)
        nc.tensor.matmul(dk2_ps[:], ones1x16[:], g2row[:], start=False,
                         stop=False)
        nc.tensor.matmul(dk2_ps[:], g2row[:], ones1x16[:], start=False,
                         stop=True)
        dk2 = big.tile([16, 16], dt.float32)
        nc.vector.tensor_scalar(out=dk2[:], in0=dk2_ps[:], scalar1=0.0,
                                scalar2=None, op0=Alu.max)
        dk = big.tile([16, 16], dt.float32)
        nc.scalar.activation(out=dk[:], in_=dk2[:], func=Act.Sqrt,
                             bias=b_eps[0:16, 0:1])
        r3 = big.tile([16, 16], dt.float32)
        nc.scalar.activation(out=r3[:], in_=dk[:], func=Act.Relu,
                             bias=b_three[0:16, 0:1], scale=-1.0)
        r3s = big.tile([16, 16], dt.float32)
        nc.scalar.activation(out=r3s[:], in_=r3[:], func=Act.Square)
        val = big.tile([16, 16], dt.float32)
        nc.scalar.activation(out=val[:], in_=r3s[:], func=Act.Ln, bias=1.0)

        nc.vector.tensor_scalar(out=val[:], in0=val[:], scalar1=valid[:, 0:1],
                                scalar2=None, op0=Alu.mult)
        vrow_ps = tiny_ps([1, 16])
        nc.tensor.transpose(vrow_ps[:], valid[:], ident16[:])
        vrow = big.tile([1, 16], dt.float32)
        nc.vector.tensor_copy(out=vrow[:], in_=vrow_ps[:])
        vbc_ps = tiny_ps([16, 16])
        nc.tensor.matmul(vbc_ps[:], ones1x16[:], vrow[:], start=True, stop=True)
        nc.vector.tensor_mul(val[:], val[:], vbc_ps[:])
        nc.vector.tensor_mul(val[:], val[:], iu16[:])

        psr = t16.tile([16, 1], dt.float32)
        nc.vector.tensor_reduce(out=psr[:], in_=val[:], axis=AX.X, op=Alu.add)
        ps_ps = tiny_ps([1, 1])
        nc.tensor.matmul(ps_ps[:], psr[:], ones16[:], start=True, stop=True)
        ps_s = t16.tile([1, 1], dt.float32)
        nc.vector.tensor_copy(out=ps_s[:], in_=ps_ps[:])

        nvm1 = t16.tile([1, 1], dt.float32)
        nc.vector.tensor_scalar(out=nvm1[:], in0=nv_s[:], scalar1=-1.0,
                                scalar2=None, op0=Alu.add)
        den = t16.tile([1, 1], dt.float32)
        nc.vector.tensor_mul(den[:], nv_s[:], nvm1[:])
        den_c = t16.tile([1, 1], dt.float32)
        nc.vector.tensor_scalar(out=den_c[:], in0=den[:], scalar1=1.0,
                                scalar2=None, op0=Alu.max)
        rdn = t16.tile([1, 1], dt.float32)
        nc.vector.reciprocal(rdn[:], den_c[:])
        lpush = t16.tile([1, 1], dt.float32)
        nc.vector.tensor_mul(lpush[:], ps_s[:], rdn[:])
        gate = t16.tile([1, 1], dt.float32)
        nc.vector.tensor_scalar(out=gate[:], in0=nv_s[:], scalar1=1.0,
                                scalar2=None, op0=Alu.is_gt)
        nc.vector.tensor_mul(lpush[:], lpush[:], gate[:])

        outt = t16.tile([1, 2], dt.float32)
        nc.vector.tensor_copy(out=outt[:, 0:1], in_=lpull[:])
        nc.vector.tensor_copy(out=outt[:, 1:2], in_=lpush[:])
        nc.sync.dma_start(out=out_d.ap(), in_=outt[:])

    nc.compile()
    return nc


def _consts(cfg):
    import ml_dtypes
    bf16 = ml_dtypes.bfloat16
    JK, JT = cfg["JK"], cfg["JT"]
    ident16 = np.eye(16, dtype=np.float32)
    iu16 = np.triu(np.ones((16, 16), np.float32), 1)
    mge1 = (np.arange(16) >= 1).astype(np.float32).reshape(16, 1)
    e6 = np.zeros((6, 48), bf16)
    for ch in range(6):
        e6[ch, ch * 8:(ch + 1) * 8] = 1
    dmask48 = np.zeros((48, 128), bf16)
    for r in range(48):
        for cc in range(128):
            if r % 8 == cc % 8:
                dmask48[r, cc] = 1
    dselk = np.zeros((JK * 5, 16 * JK), bf16)
    for J in range(JK):
        dselk[J * 5:(J + 1) * 5, J::JK] = 1
    c5 = np.zeros((JK * 5, 5), np.float32)
    for J in range(JK):
        for c in range(5):
            c5[J * 5 + c, c] = 1
    dselt = np.zeros((JT * 2, 16 * JT), bf16)
    for J in range(JT):
        dselt[J * 2:(J + 1) * 2, J::JT] = 1
    c2 = np.zeros((JT * 2, 2), np.float32)
    for J in range(JT):
        for c in range(2):
            c2[J * 2 + c, c] = 1
    return dict(ident16=ident16, iu16=iu16, mge1=mge1, e6=e6,
                dmask48=dmask48, dselk=dselk, c5=c5, dselt=dselt, c2=c2)


def make_in_maps(outputs, gt_texts, gt_kernels, cfg):
    P, F = cfg["P"], cfg["F"]
    B = outputs.shape[0]
    consts = _consts(cfg)
    in_maps = []
    for b in range(B):
        sim = np.ascontiguousarray(outputs[b, 4:8], dtype=np.float32)
        in_maps.append(dict(
            sim=sim.reshape(4, P, F),
            kt=np.ascontiguousarray(gt_kernels[b], dtype=np.int32).reshape(P, F),
            tt=np.ascontiguousarray(gt_texts[b], dtype=np.int32).reshape(P, F),
            **consts,
        ))
    return in_maps


def kernel(outputs, gt_texts, gt_kernels, gt_tops=None, gt_bots=None):
    from concourse import bass_utils
    outputs = np.asarray(outputs)
    gt_texts = np.asarray(gt_texts)
    gt_kernels = np.asarray(gt_kernels)
    B = outputs.shape[0]
    cfg = _cfg(outputs.shape[2], outputs.shape[3])
    key = (cfg["H"], cfg["W"])
    if key not in _CACHE:
        _CACHE[key] = build(cfg, for_sim=False)
    nc = _CACHE[key]
    in_maps = make_in_maps(outputs, gt_texts, gt_kernels, cfg)
    res = bass_utils.run_bass_kernel_spmd(nc, in_maps, core_ids=list(range(B)))
    lpull = np.array([res.results[b]["out"][0, 0] for b in range(B)], np.float32)
    lpush = np.array([res.results[b]["out"][0, 1] for b in range(B)], np.float32)
    return lpull, lpush



# revision 3
# speedup vs baseline: 51.5948x; 51.5948x over previous
"""Trainium2 Bass kernel for nn_BoundLoss (pull/push embedding loss, segment_reduce).

Strategy: pure data parallel, 1 image per NeuronCore (B=8, 8 cores).
All reductions on device. Output per core: (loss_pull, loss_push) scalars.

Key ideas:
  - Segment sums (by gt_kernels / gt_texts, M=16 ids) via block-diagonal
    one-hot matmuls on the tensor engine: J pixel-column groups share one
    stationary-weight load; off-diagonal products land in PSUM cells we
    never read.
  - The per-pixel gather of centroid stats G[tt[n]] is folded into a single
    stationary-weight matmul computing, for every pixel and every id m,
    z_m = s2 - 2*dot(sim, G[m]) + g2[m]  (a "D-hat" tensor), using a
    block-diagonal G-matrix with 8 pixel sub-row slots; per-pixel selection
    of the right m is 16 mask-mult-accumulate passes on the vector engine.
  - l = log1p(relu(sqrt(z)-0.5)^2) chain on the scalar engine.
"""

import os
import numpy as np
from contextlib import ExitStack

EPS = 1e-12

FULL_CFG = dict(H=640, W=640)

_CACHE = {}


def _cfg(H, W):
    P = 128
    N = H * W
    F = N // P
    assert F * P == N
    if F % 400 == 0 and F >= 1600:
        FC = 400
    else:
        FC = F // 4 if F % 4 == 0 and F // 4 <= 512 else F
        if FC > 512:
            raise ValueError("bad FC")
    NQ = max(1, F // (2 * FC))
    FQ = F // NQ
    assert FQ % FC == 0 and F % FQ == 0
    OHC = min(F, 400)
    assert F % OHC == 0
    # PE group sizes (pixel columns per stationary-weight load)
    JK = 25 if OHC % 25 == 0 else 8   # kt family: 5 ch -> 125 weight cols
    JT = 32 if FQ % 32 == 0 else 8    # tt family: 2 ch -> 64 weight cols
    assert OHC % JK == 0 and FQ % JT == 0
    return dict(H=H, W=W, P=P, N=N, F=F, FC=FC, FQ=FQ, NQ=NQ, OHC=OHC,
                JK=JK, JT=JT, M=16)


def build(cfg, for_sim=False, repeat=1):
    import concourse.bass as bass
    import concourse.bacc as bacc
    import concourse.tile as tile
    from concourse import mybir

    dt = mybir.dt
    Alu = mybir.AluOpType
    Act = mybir.ActivationFunctionType
    AX = mybir.AxisListType

    P, F, M = cfg["P"], cfg["F"], cfg["M"]
    FC, FQ, NQ, OHC = cfg["FC"], cfg["FQ"], cfg["NQ"], cfg["OHC"]
    JK, JT = cfg["JK"], cfg["JT"]

    nc = bacc.Bacc("TRN2", target_bir_lowering=False, debug=for_sim)

    sim_d = nc.dram_tensor("sim", [4, P, F], dt.float32, kind="ExternalInput")
    kt_d = nc.dram_tensor("kt", [P, F], dt.int32, kind="ExternalInput")
    tt_d = nc.dram_tensor("tt", [P, F], dt.int32, kind="ExternalInput")
    ident_d = nc.dram_tensor("ident16", [16, 16], dt.float32, kind="ExternalInput")
    iu_d = nc.dram_tensor("iu16", [16, 16], dt.float32, kind="ExternalInput")
    mge1_d = nc.dram_tensor("mge1", [16, 1], dt.float32, kind="ExternalInput")
    e6_d = nc.dram_tensor("e6", [6, 48], dt.bfloat16, kind="ExternalInput")
    dmask_d = nc.dram_tensor("dmask48", [48, 128], dt.bfloat16,
                             kind="ExternalInput")
    dselk_d = nc.dram_tensor("dselk", [JK * 5, 16 * JK], dt.bfloat16,
                             kind="ExternalInput")
    c5_d = nc.dram_tensor("c5", [JK * 5, 5], dt.float32, kind="ExternalInput")
    dselt_d = nc.dram_tensor("dselt", [JT * 2, 16 * JT], dt.bfloat16,
                             kind="ExternalInput")
    c2_d = nc.dram_tensor("c2", [JT * 2, 2], dt.float32, kind="ExternalInput")
    out_d = nc.dram_tensor("out", [1, 2], dt.float32, kind="ExternalOutput")

    with ExitStack() as octx:
        tc = octx.enter_context(tile.TileContext(nc, trace_sim=for_sim))
        for _rep in range(repeat):
            _body(cfg, nc, tc, bass, mybir, sim_d, kt_d, tt_d, ident_d, iu_d,
                  mge1_d, e6_d, dmask_d, dselk_d, c5_d, dselt_d, c2_d, out_d,
                  _rep)

    nc.compile()
    return nc


def _body(cfg, nc, tc, bass, mybir, sim_d, kt_d, tt_d, ident_d, iu_d, mge1_d,
          e6_d, dmask_d, dselk_d, c5_d, dselt_d, c2_d, out_d, rep):
    dt = mybir.dt
    Alu = mybir.AluOpType
    Act = mybir.ActivationFunctionType
    AX = mybir.AxisListType

    P, F, M = cfg["P"], cfg["F"], cfg["M"]
    FC, FQ, NQ, OHC = cfg["FC"], cfg["FQ"], cfg["NQ"], cfg["OHC"]
    JK, JT = cfg["JK"], cfg["JT"]
    R = f"r{rep}"

    with ExitStack() as ctx:

        big = ctx.enter_context(tc.tile_pool(name="big" + R, bufs=1))
        t16 = ctx.enter_context(tc.tile_pool(name="t16" + R, bufs=1))
        pst = ctx.enter_context(tc.tile_pool(name="pst" + R, bufs=2, space="PSUM"))

        _tiny_n = [0]

        def tiny_ps(shape):
            _tiny_n[0] += 1
            return pst.tile(shape, dt.float32, tag="tiny",
                            name=f"tinyps{_tiny_n[0]}")

        # ---- constants ----
        ident16 = big.tile([16, 16], dt.float32)
        nc.sync.dma_start(out=ident16[:], in_=ident_d.ap())
        iu16 = big.tile([16, 16], dt.float32)
        nc.sync.dma_start(out=iu16[:], in_=iu_d.ap())
        mge1 = t16.tile([16, 1], dt.float32)
        nc.sync.dma_start(out=mge1[:], in_=mge1_d.ap())
        e6 = big.tile([6, 48], dt.bfloat16)
        nc.sync.dma_start(out=e6[:], in_=e6_d.ap())
        dmask48 = big.tile([48, 128], dt.bfloat16)
        nc.sync.dma_start(out=dmask48[:], in_=dmask_d.ap())
        dselk = big.tile([JK * 5, 16 * JK], dt.bfloat16)
        nc.sync.dma_start(out=dselk[:], in_=dselk_d.ap())
        c5 = big.tile([JK * 5, 5], dt.float32)
        nc.sync.dma_start(out=c5[:], in_=c5_d.ap())
        dselt = big.tile([JT * 2, 16 * JT], dt.bfloat16)
        nc.sync.dma_start(out=dselt[:], in_=dselt_d.ap())
        c2 = big.tile([JT * 2, 2], dt.float32)
        nc.sync.dma_start(out=c2[:], in_=c2_d.ap())
        ones16 = t16.tile([16, 1], dt.float32)
        nc.vector.memset(ones16[:], 1.0)
        b_eps = t16.tile([128, 1], dt.float32)
        nc.vector.memset(b_eps[:], EPS)
        b_mhalf = t16.tile([128, 1], dt.float32)
        nc.vector.memset(b_mhalf[:], -0.5)
        b_three = t16.tile([128, 1], dt.float32)
        nc.vector.memset(b_three[:], 3.0)

        # ---- persistent planes ----
        vplI = big.tile([P, F, 5], dt.bfloat16)   # (sim0..3, ones) interleaved
        simbf4 = big.tile([P, 4, F], dt.bfloat16)  # plane-major (for remaps)
        vpl2I = big.tile([P, F, 2], dt.bfloat16)  # (l, ones) interleaved
        s2bf = big.tile([P, F], dt.bfloat16)
        ttbf = big.tile([P, F], dt.bfloat16)
        gblock = big.tile([48, 128], dt.bfloat16)
        ktp_ctx = ExitStack()
        ktp = ktp_ctx.enter_context(tc.tile_pool(name="ktp" + R, bufs=1))
        ktbf = ktp.tile([P, F], dt.bfloat16)

        nc.gpsimd.memset(vplI[:, :, 4], 1.0)
        nc.gpsimd.memset(vpl2I[:, :, 1], 1.0)

        # ---- load + convert inputs; s2 ----
        with tc.tile_pool(name="ld" + R, bufs=2) as ld:
            kti = ld.tile([P, F], dt.int32, tag="idx")
            nc.sync.dma_start(out=kti[:], in_=kt_d.ap())
            nc.vector.tensor_copy(out=ktbf[:], in_=kti[:])
            tti = ld.tile([P, F], dt.int32, tag="idx")
            nc.sync.dma_start(out=tti[:], in_=tt_d.ap())
            nc.vector.tensor_copy(out=ttbf[:], in_=tti[:])
            for c in range(4):
                sf = ld.tile([P, F], dt.float32, tag="simf")
                nc.sync.dma_start(out=sf[:], in_=sim_d.ap()[c])
                nc.vector.tensor_copy(out=simbf4[:, c, :], in_=sf[:])
                eng = nc.gpsimd if c % 2 == 0 else nc.vector
                eng.tensor_copy(out=vplI[:, :, c], in_=sf[:])
                if c == 0:
                    nc.scalar.activation(out=s2bf[:], in_=sf[:], func=Act.Square)
                else:
                    sq = ld.tile([P, F], dt.bfloat16, tag="sq")
                    nc.scalar.activation(out=sq[:], in_=sf[:], func=Act.Square)
                    nc.vector.tensor_add(s2bf[:], s2bf[:], sq[:])

        # ---- phase 1: kernel-id segment sums ----
        # stationary = vplI group [128, JK*5] (contiguous); moving = onehot
        # [128, (m, J')]; psum [(J,c), (m,J')]; diagonal J==J' is wanted.
        NGK = F // JK
        skC_ps = tiny_ps([5, 16 * JK])
        with tc.tile_pool(name="ps1" + R, bufs=1, space="PSUM") as psum1:
            p1 = psum1.tile([JK * 5, 16 * JK], dt.float32)
            with tc.tile_pool(name="ohk" + R, bufs=2) as ohkp:
                for blk in range(F // OHC):
                    ohk = ohkp.tile([P, M, OHC], dt.bfloat16, tag="ohk")
                    for m in range(M):
                        nc.vector.tensor_scalar(
                            out=ohk[:, m, :],
                            in0=ktbf[:, blk * OHC:(blk + 1) * OHC],
                            scalar1=float(m), scalar2=None, op0=Alu.is_equal)
                    for t in range(OHC // JK):
                        g = blk * OHC + t * JK
                        lhsT = vplI[:, g:g + JK, :]
                        rhs = ohk[:, :, t * JK:(t + 1) * JK]
                        nc.tensor.matmul(p1[:], lhsT, rhs,
                                         start=(g == 0), stop=(g == F - JK))
            # diagonal extraction: mask off-diag, sum rows per channel via
            # matmul, then strided-reduce over J'.
            p1m = big.tile([JK * 5, 16 * JK], dt.float32)
            nc.vector.tensor_mul(p1m[:], p1[:], dselk[:])
            nc.tensor.matmul(skC_ps[:], c5[:], p1m[:], start=True, stop=True)
        skC = big.tile([5, 16], dt.float32)
        nc.vector.tensor_reduce(
            out=skC[:],
            in_=skC_ps.rearrange("p (m j) -> p m j", j=JK),
            axis=AX.X, op=Alu.add)
        skT_ps = tiny_ps([16, 5])
        nc.tensor.transpose(skT_ps[:], skC[:], ident16[0:5, 0:5])
        sk = big.tile([16, 5], dt.float32)
        nc.vector.tensor_copy(out=sk[:], in_=skT_ps[:])
        ktp_ctx.close()

        # ---- stats: G, g2, valid, Ghat-block ----
        cntk_c = t16.tile([16, 1], dt.float32)
        nc.vector.tensor_scalar(out=cntk_c[:], in0=sk[:, 4:5], scalar1=1.0,
                                scalar2=None, op0=Alu.max)
        rck = t16.tile([16, 1], dt.float32)
        nc.vector.reciprocal(rck[:], cntk_c[:])
        G = t16.tile([16, 4], dt.float32)
        nc.vector.tensor_scalar(out=G[:], in0=sk[:, 0:4], scalar1=rck[:, 0:1],
                                scalar2=None, op0=Alu.mult)
        gsq = t16.tile([16, 4], dt.float32)
        nc.scalar.activation(out=gsq[:], in_=G[:], func=Act.Square)
        g2 = t16.tile([16, 1], dt.float32)
        nc.vector.tensor_reduce(out=g2[:], in_=gsq[:], axis=AX.X, op=Alu.add)

        gext = t16.tile([16, 6], dt.float32)
        nc.vector.tensor_scalar(out=gext[:, 0:4], in0=G[:], scalar1=-2.0,
                                scalar2=None, op0=Alu.mult)
        nc.vector.tensor_copy(out=gext[:, 4:5], in_=g2[:])
        nc.vector.memset(gext[:, 5:6], 1.0)

        gsT = tiny_ps([6, 16])
        nc.tensor.transpose(gsT[:], gext[:], ident16[:])
        gsb = big.tile([6, 16], dt.bfloat16)
        nc.vector.tensor_copy(out=gsb[:], in_=gsT[:])
        # gblock[ch*8+i, m*8+i'] = delta(i,i') * Ghat[ch, m]:
        # broadcast Ghat across 8 col-slots (DVE), expand partition groups
        # via E6 matmul, then mask the slot diagonal.
        gbig_row = big.tile([6, 128], dt.bfloat16)
        ga = gsb[:]
        gbc_ap = bass.AP(tensor=ga.tensor, offset=ga.offset,
                         ap=[list(ga.ap[0]), list(ga.ap[1]), [0, 8]])
        nc.vector.tensor_copy(out=gbig_row[:], in_=gbc_ap)
        gbig_ps = tiny_ps([48, 128])
        nc.tensor.matmul(gbig_ps[:], e6[:], gbig_row[:], start=True, stop=True)
        nc.vector.tensor_mul(gblock[:], gbig_ps[:], dmask48[:])

        vg = t16.tile([16, 1], dt.float32)
        nc.vector.tensor_scalar(out=vg[:], in0=sk[:, 4:5], scalar1=0.0,
                                scalar2=None, op0=Alu.is_gt)
        valid = t16.tile([16, 1], dt.float32)
        nc.vector.tensor_mul(valid[:], vg[:], mge1[:])

        # ---- phase 2: per-quarter D-hat, resolve, l, PE-tt ----
        NGT = FQ // JT
        psum3 = ctx.enter_context(tc.tile_pool(name="ps3" + R, bufs=1, space="PSUM"))
        p3 = psum3.tile([JT * 2, 16 * JT], dt.float32)
        onesq = big.tile([P, FQ], dt.bfloat16)
        nc.gpsimd.memset(onesq[:], 1.0)
        zfull = big.tile([P, F], dt.bfloat16)
        # ---- loop 1: D-hat + resolve -> zfull ----
        with tc.tile_pool(name="q_sb" + R, bufs=1) as qp, \
             tc.tile_pool(name="q_sb2" + R, bufs=2) as qp2, \
             tc.tile_pool(name="q_ps" + R, bufs=3, space="PSUM") as qps, \
             tc.tile_pool(name="q_z" + R, bufs=2) as zp:
            for q in range(NQ):
                qlo = q * FQ
                dstage = qp.tile([P, 16, FQ], dt.bfloat16, tag="dstage")
                for sh in range(FQ // FC):
                    slo = qlo + sh * FC
                    # rhs layout [48 = ch*8+i, 16u, FC]
                    rhs_h = qp2.tile([48, 16, FC], dt.bfloat16, tag="rhs")
                    for ch in range(6):
                        src = (simbf4[:, ch, slo:slo + FC] if ch < 4 else
                               (onesq[:, 0:FC] if ch == 4 else
                                s2bf[:, slo:slo + FC]))
                        nc.sync.dma_start(
                            out=rhs_h[ch * 8:(ch + 1) * 8, :, :], in_=src)
                    for u in range(16):
                        pd = qps.tile([128, FC], dt.float32, tag="pd")
                        nc.tensor.matmul(pd[:], gblock[:], rhs_h[:, u, :],
                                         start=True, stop=True)
                        nc.scalar.activation(
                            out=dstage[:, u, sh * FC:(sh + 1) * FC],
                            in_=pd[:], func=Act.Copy)
                dq = qp.tile([P, M, FQ], dt.bfloat16, tag="dq")
                for m in range(M):
                    nc.sync.dma_start(out=dq[:, m, :],
                                      in_=dstage[m * 8:(m + 1) * 8, :, :])

                # z = sum_m (tt==m) * dq_m  (disjoint masks, fused is_equal)
                zwin = zfull[:, qlo:qlo + FQ]
                twin = ttbf[:, qlo:qlo + FQ]
                nc.vector.scalar_tensor_tensor(
                    out=zwin, in0=twin, scalar=0.0, in1=dq[:, 0, :],
                    op0=Alu.is_equal, op1=Alu.mult)
                for m in range(1, M):
                    pm_t = zp.tile([P, FQ], dt.bfloat16, tag="pm")
                    nc.vector.scalar_tensor_tensor(
                        out=pm_t[:], in0=twin, scalar=float(m),
                        in1=dq[:, m, :], op0=Alu.is_equal, op1=Alu.mult)
                    nc.vector.tensor_add(zwin, zwin, pm_t[:])

        # ---- loop 2: masks, l-chain, PE-tt ----
        with tc.tile_pool(name="l2_sb" + R, bufs=2) as l2p, \
             tc.tile_pool(name="l2_z" + R, bufs=2) as zp2:
            for q in range(NQ):
                qlo = q * FQ
                ohtt = l2p.tile([P, M, FQ], dt.bfloat16, tag="ohtt")
                for m in range(M):
                    nc.vector.tensor_scalar(
                        out=ohtt[:, m, :], in0=ttbf[:, qlo:qlo + FQ],
                        scalar1=float(m), scalar2=None, op0=Alu.is_equal)

                t0 = zp2.tile([P, FQ], dt.bfloat16, tag="lt0")
                nc.scalar.activation(out=t0[:], in_=zfull[:, qlo:qlo + FQ],
                                     func=Act.Relu)
                t1 = zp2.tile([P, FQ], dt.bfloat16, tag="lt1")
                nc.scalar.activation(out=t1[:], in_=t0[:], func=Act.Sqrt,
                                     bias=b_eps[:, 0:1])
                t2 = zp2.tile([P, FQ], dt.bfloat16, tag="lt0")
                nc.scalar.activation(out=t2[:], in_=t1[:], func=Act.Relu,
                                     bias=b_mhalf[:, 0:1])
                t3 = zp2.tile([P, FQ], dt.bfloat16, tag="lt1")
                nc.scalar.activation(out=t3[:], in_=t2[:], func=Act.Square)
                nc.scalar.activation(out=vpl2I[:, qlo:qlo + FQ, 0], in_=t3[:],
                                     func=Act.Ln, bias=1.0)

                for t in range(NGT):
                    g = qlo + t * JT
                    lhsT = vpl2I[:, g:g + JT, :]
                    rhs = ohtt[:, :, t * JT:(t + 1) * JT]
                    nc.tensor.matmul(p3[:], lhsT, rhs,
                                     start=(g == 0), stop=(g == F - JT))

        # tt-family diagonal extraction (same scheme as phase 1)
        p3m = big.tile([JT * 2, 16 * JT], dt.float32)
        nc.vector.tensor_mul(p3m[:], p3[:], dselt[:])
        stC_ps = tiny_ps([2, 16 * JT])
        nc.tensor.matmul(stC_ps[:], c2[:], p3m[:], start=True, stop=True)
        stC = big.tile([2, 16], dt.float32)
        nc.vector.tensor_reduce(
            out=stC[:],
            in_=stC_ps.rearrange("p (m j) -> p m j", j=JT),
            axis=AX.X, op=Alu.add)
        stT_ps = tiny_ps([16, 2])
        nc.tensor.transpose(stT_ps[:], stC[:], ident16[0:2, 0:2])
        st = big.tile([16, 2], dt.float32)
        nc.vector.tensor_copy(out=st[:], in_=stT_ps[:])

        # ---- pull loss ----
        cntt_c = t16.tile([16, 1], dt.float32)
        nc.vector.tensor_scalar(out=cntt_c[:], in0=st[:, 1:2], scalar1=1.0,
                                scalar2=None, op0=Alu.max)
        rct = t16.tile([16, 1], dt.float32)
        nc.vector.reciprocal(rct[:], cntt_c[:])
        pim = t16.tile([16, 1], dt.float32)
        nc.vector.tensor_mul(pim[:], st[:, 0:1], rct[:])
        nc.vector.tensor_mul(pim[:], pim[:], valid[:])

        num_ps = tiny_ps([1, 1])
        nc.tensor.matmul(num_ps[:], pim[:], ones16[:], start=True, stop=True)
        nv_ps = tiny_ps([1, 1])
        nc.tensor.matmul(nv_ps[:], valid[:], ones16[:], start=True, stop=True)
        num_s = t16.tile([1, 1], dt.float32)
        nc.vector.tensor_copy(out=num_s[:], in_=num_ps[:])
        nv_s = t16.tile([1, 1], dt.float32)
        nc.vector.tensor_copy(out=nv_s[:], in_=nv_ps[:])

        nv_c = t16.tile([1, 1], dt.float32)
        nc.vector.tensor_scalar(out=nv_c[:], in0=nv_s[:], scalar1=1.0,
                                scalar2=None, op0=Alu.max)
        rnv = t16.tile([1, 1], dt.float32)
        nc.vector.reciprocal(rnv[:], nv_c[:])
        lpull = t16.tile([1, 1], dt.float32)
        nc.vector.tensor_mul(lpull[:], num_s[:], rnv[:])

        # ---- push loss ----
        ones1x16 = big.tile([1, 16], dt.float32)
        nc.vector.memset(ones1x16[:], 1.0)
        gT_ps = tiny_ps([4, 16])
        nc.tensor.transpose(gT_ps[:], G[:], ident16[:])
        gt_sb = big.tile([4, 16], dt.float32)
        nc.vector.tensor_copy(out=gt_sb[:], in_=gT_ps[:])
        g2r_ps = tiny_ps([1, 16])
        nc.tensor.transpose(g2r_ps[:], g2[:], ident16[:])
        g2row = big.tile([1, 16], dt.float32)
        nc.vector.tensor_copy(out=g2row[:], in_=g2r_ps[:])
        mgt2 = big.tile([4, 16], dt.float32)
        nc.vector.tensor_scalar(out=mgt2[:], in0=gt_sb[:], scalar1=-2.0,
                                scalar2=None, op0=Alu.mult)
        dk2_ps = tiny_ps([16, 16])
        nc.tensor.matmul(dk2_ps[:], mgt2[:], gt_sb[:], start=True, stop=False)
        nc.tensor.matmul(dk2_ps[:], ones1x16[:], g2row[:], start=False,
                         stop=False)
        nc.tensor.matmul(dk2_ps[:], g2row[:], ones1x16[:], start=False,
                         stop=True)
        dk2 = big.tile([16, 16], dt.float32)
        nc.vector.tensor_scalar(out=dk2[:], in0=dk2_ps[:], scalar1=0.0,
                                scalar2=None, op0=Alu.max)
        dk = big.tile([16, 16], dt.float32)
        nc.scalar.activation(out=dk[:], in_=dk2[:], func=Act.Sqrt,
                             bias=b_eps[0:16, 0:1])
        r3 = big.tile([16, 16], dt.float32)
        nc.scalar.activation(out=r3[:], in_=dk[:], func=Act.Relu,
                             bias=b_three[0:16, 0:1], scale=-1.0)
        r3s = big.tile([16, 16], dt.float32)
        nc.scalar.activation(out=r3s[:], in_=r3[:], func=Act.Square)
        val = big.tile([16, 16], dt.float32)
        nc.scalar.activation(out=val[:], in_=r3s[:], func=Act.Ln, bias=1.0)

        nc.vector.tensor_scalar(out=val[:], in0=val[:], scalar1=valid[:, 0:1],
                                scalar2=None, op0=Alu.mult)
        vrow_ps = tiny_ps([1, 16])
        nc.tensor.transpose(vrow_ps[:], valid[:], ident16[:])
        vrow = big.tile([1, 16], dt.float32)
        nc.vector.tensor_copy(out=vrow[:], in_=vrow_ps[:])
        vbc_ps = tiny_ps([16, 16])
        nc.tensor.matmul(vbc_ps[:], ones1x16[:], vrow[:], start=True, stop=True)
        nc.vector.tensor_mul(val[:], val[:], vbc_ps[:])
        nc.vector.tensor_mul(val[:], val[:], iu16[:])

        psr = t16.tile([16, 1], dt.float32)
        nc.vector.tensor_reduce(out=psr[:], in_=val[:], axis=AX.X, op=Alu.add)
        ps_ps = tiny_ps([1, 1])
        nc.tensor.matmul(ps_ps[:], psr[:], ones16[:], start=True, stop=True)
        ps_s = t16.tile([1, 1], dt.float32)
        nc.vector.tensor_copy(out=ps_s[:], in_=ps_ps[:])

        nvm1 = t16.tile([1, 1], dt.float32)
        nc.vector.tensor_scalar(out=nvm1[:], in0=nv_s[:], scalar1=-1.0,
                                scalar2=None, op0=Alu.add)
        den = t16.tile([1, 1], dt.float32)
        nc.vector.tensor_mul(den[:], nv_s[:], nvm1[:])
        den_c = t16.tile([1, 1], dt.float32)
        nc.vector.tensor_scalar(out=den_c[:], in0=den[:], scalar1=1.0,
                                scalar2=None, op0=Alu.max)
        rdn = t16.tile([1, 1], dt.float32)
        nc.vector.reciprocal(rdn[:], den_c[:])
        lpush = t16.tile([1, 1], dt.float32)
        nc.vector.tensor_mul(lpush[:], ps_s[:], rdn[:])
        gate = t16.tile([1, 1], dt.float32)
        nc.vector.tensor_scalar(out=gate[:], in0=nv_s[:], scalar1=1.0,
                                scalar2=None, op0=Alu.is_gt)
        nc.vector.tensor_mul(lpush[:], lpush[:], gate[:])

        outt = t16.tile([1, 2], dt.float32)
        nc.vector.tensor_copy(out=outt[:, 0:1], in_=lpull[:])
        nc.vector.tensor_copy(out=outt[:, 1:2], in_=lpush[:])
        nc.sync.dma_start(out=out_d.ap(), in_=outt[:])


def _consts(cfg):
    import ml_dtypes
    bf16 = ml_dtypes.bfloat16
    JK, JT = cfg["JK"], cfg["JT"]
    ident16 = np.eye(16, dtype=np.float32)
    iu16 = np.triu(np.ones((16, 16), np.float32), 1)
    mge1 = (np.arange(16) >= 1).astype(np.float32).reshape(16, 1)
    e6 = np.zeros((6, 48), bf16)
    for ch in range(6):
        e6[ch, ch * 8:(ch + 1) * 8] = 1
    dmask48 = np.zeros((48, 128), bf16)
    for r in range(48):
        for cc in range(128):
            if r % 8 == cc % 8:
                dmask48[r, cc] = 1
    dselk = np.zeros((JK * 5, 16 * JK), bf16)
    for J in range(JK):
        dselk[J * 5:(J + 1) * 5, J::JK] = 1
    c5 = np.zeros((JK * 5, 5), np.float32)
    for J in range(JK):
        for c in range(5):
            c5[J * 5 + c, c] = 1
    dselt = np.zeros((JT * 2, 16 * JT), bf16)
    for J in range(JT):
        dselt[J * 2:(J + 1) * 2, J::JT] = 1
    c2 = np.zeros((JT * 2, 2), np.float32)
    for J in range(JT):
        for c in range(2):
            c2[J * 2 + c, c] = 1
    return dict(ident16=ident16, iu16=iu16, mge1=mge1, e6=e6,
                dmask48=dmask48, dselk=dselk, c5=c5, dselt=dselt, c2=c2)


def make_in_maps(outputs, gt_texts, gt_kernels, cfg):
    P, F = cfg["P"], cfg["F"]
    B = outputs.shape[0]
    consts = _consts(cfg)
    in_maps = []
    for b in range(B):
        sim = np.ascontiguousarray(outputs[b, 4:8], dtype=np.float32)
        in_maps.append(dict(
            sim=sim.reshape(4, P, F),
            kt=np.ascontiguousarray(gt_kernels[b], dtype=np.int32).reshape(P, F),
            tt=np.ascontiguousarray(gt_texts[b], dtype=np.int32).reshape(P, F),
            **consts,
        ))
    return in_maps


def kernel(outputs, gt_texts, gt_kernels, gt_tops=None, gt_bots=None):
    from concourse import bass_utils
    outputs = np.asarray(outputs)
    gt_texts = np.asarray(gt_texts)
    gt_kernels = np.asarray(gt_kernels)
    B = outputs.shape[0]
    cfg = _cfg(outputs.shape[2], outputs.shape[3])
    key = (cfg["H"], cfg["W"])
    if key not in _CACHE:
        _CACHE[key] = build(cfg, for_sim=False)
    nc = _CACHE[key]
    in_maps = make_in_maps(outputs, gt_texts, gt_kernels, cfg)
    res = bass_utils.run_bass_kernel_spmd(nc, in_maps, core_ids=list(range(B)))
    lpull = np.array([res.results[b]["out"][0, 0] for b in range(B)], np.float32)
    lpush = np.array([res.results[b]["out"][0, 1] for b in range(B)], np.float32)
    return lpull, lpush

